# revision 7
# baseline (speedup 1.0000x reference)
"""Dcls1d (dilated conv with learnable spacings) on 8 Trainium2 NeuronCores.

Problem (hardcoded): input [32, 256, 4096] f32, weight [256, 256, 7] f32,
P [256, 256, 7] f32, bias [256] f32 -> output [32, 256, 4097] f32.
The 7 taps are scattered at continuous positions into a dense 56-wide
kernel with linear interpolation (done host-side, bit-identical to the
reference fp32 math), then the dense conv runs on-device as 56 shifted
[128x128]x[128x512] fp32r matmuls accumulating in PSUM.

Sharding: data-parallel over batch — each of the 8 cores gets 4 batches,
weights/bias broadcast. No collectives; outputs concatenated on host.
"""

import os
from contextlib import ExitStack

import numpy as np

import concourse.bacc as bacc
import concourse.mybir as mybir
import concourse.tile as tile
from concourse.bass_utils import run_bass_kernel_spmd

DT = mybir.dt

B, CIN, COUT, L = 32, 256, 256, 4096
KTAPS, DIL, PAD = 7, 8, 28
LD = KTAPS * DIL  # 56 dense kernel width
TOUT = L + 2 * PAD - LD + 1  # 4097
NCORES = 8
BPC = B // NCORES  # batches per core

NT = 512  # output cols per psum tile
NCHUNK = L // NT  # 8 full chunks
CH_W = NT + LD  # 568 xpad cols per chunk

_nc_cache = {}


def build_dense_kernel(weight: np.ndarray, P: np.ndarray) -> np.ndarray:
    """Scatter taps into dense [O, C, LD] kernel. Replicates the reference's
    fp32 arithmetic exactly (clip/floor/frac all in float32)."""
    w = weight.astype(np.float32)
    pos = np.clip(P.astype(np.float32) + np.float32(LD // 2), np.float32(0.0), np.float32(LD - 1))
    lo = np.floor(pos)
    frac = pos - lo
    lo_i = lo.astype(np.int64)
    hi_i = np.minimum(lo_i + 1, LD - 1)
    O, C, K = w.shape
    oi = np.arange(O)[:, None, None]
    ci = np.arange(C)[None, :, None]
    D = np.zeros((O, C, LD), np.float32)
    np.add.at(D, (oi, ci, lo_i), w * (np.float32(1.0) - frac))
    np.add.at(D, (oi, ci, hi_i), w * frac)
    return D


def build_nc(bpc=BPC, nchunk=NCHUNK):
    """Per-core program: conv of [bpc, 256, L'] with dense kernel."""
    Lc = nchunk * NT
    tout = Lc + 1 if nchunk == NCHUNK else Lc  # tail col only for full length

    nc = bacc.Bacc("TRN2", target_bir_lowering=False, debug=False)
    x_d = nc.dram_tensor("x", [bpc, CIN, Lc], DT.float32, kind="ExternalInput").ap()
    dw_d = nc.dram_tensor("dw", [128, LD, 2, 2, 128], DT.float32, kind="ExternalInput").ap()
    bias_d = nc.dram_tensor("bias", [128, 2], DT.float32, kind="ExternalInput").ap()
    zp_d = nc.dram_tensor("zp", [128, 2, PAD], DT.float32, kind="ExternalInput").ap()
    y_d = nc.dram_tensor("y", [bpc, COUT, tout], DT.float32, kind="ExternalOutput").ap()

    with ExitStack() as ctx:
        tc = ctx.enter_context(tile.TileContext(nc))
        wpool = ctx.enter_context(tc.tile_pool(name="w", bufs=1))
        xpool = ctx.enter_context(tc.tile_pool(name="x", bufs=3))
        opool = ctx.enter_context(tc.tile_pool(name="o", bufs=4))
        cpool = ctx.enter_context(tc.tile_pool(name="c", bufs=1))
        pspool = ctx.enter_context(tc.tile_pool(name="ps", bufs=4, space="PSUM"))
        tpool = ctx.enter_context(tc.tile_pool(name="tl", bufs=2, space="PSUM"))

        dw_t = wpool.tile([128, LD, 2, 2, 128], DT.float32r)
        nc.sync.dma_start(dw_t[:], dw_d[:].bitcast(DT.float32r))
        bias_t = cpool.tile([128, 2], DT.float32)
        nc.sync.dma_start(bias_t[:], bias_d[:])
        # tail-column staging: x cols needed for output col t=Lc across batches
        xtail = cpool.tile([128, 2, PAD, bpc], DT.float32r)

        for b in range(bpc):
            for j in range(nchunk):
                t0 = NT * j
                # chunk holds xpad cols [t0, t0+CH_W); real data occupies
                # chunk-local [lo_real, hi_real), zeros DMA'd outside it
                lo_real = PAD if j == 0 else 0
                hi_real = min(CH_W, Lc - t0 + PAD)
                xc = xpool.tile([128, 2, CH_W], DT.float32r)
                if lo_real:
                    nc.sync.dma_start(
                        xc[:, :, 0:lo_real], zp_d[:].bitcast(DT.float32r)
                    )
                if hi_real < CH_W:
                    nc.sync.dma_start(
                        xc[:, :, hi_real:CH_W], zp_d[:].bitcast(DT.float32r)
                    )
                for ct in range(2):
                    nc.sync.dma_start(
                        xc[:, ct, lo_real:hi_real],
                        x_d[
                            b,
                            ct * 128 : (ct + 1) * 128,
                            t0 - PAD + lo_real : t0 - PAD + hi_real,
                        ].bitcast(DT.float32r),
                    )
                for ot in range(2):
                    ps = pspool.tile([128, NT], DT.float32)
                    n_acc = LD * 2
                    i = 0
                    for d in range(LD):
                        for ct in range(2):
                            nc.tensor.matmul(
                                ps[:],
                                dw_t[:, d, ct, ot, :],
                                xc[:, ct, d : d + NT],
                                start=(i == 0),
                                stop=(i == n_acc - 1),
                            )
                            i += 1
                    ob = opool.tile([128, NT], DT.float32)
                    nc.vector.tensor_scalar_add(ob[:], ps[:], bias_t[:, ot : ot + 1])
                    nc.sync.dma_start(
                        y_d[b, ot * 128 : (ot + 1) * 128, t0 : t0 + NT], ob[:]
                    )
                if tout != Lc and j == nchunk - 1:
                    # stash x cols [Lc-PAD, Lc) (chunk-local [NT, NT+PAD)) for tail
                    nc.vector.tensor_copy(
                        xtail[:, :, :, b], xc[:, :, NT : NT + PAD]
                    )

        if tout != Lc:
            # output col t=Lc: only taps d < PAD see real data (rest padding)
            for ot in range(2):
                pst = tpool.tile([128, bpc], DT.float32)
                n_acc = PAD * 2
                i = 0
                for d in range(PAD):
                    for ct in range(2):
                        nc.tensor.matmul(
                            pst[:],
                            dw_t[:, d, ct, ot, :],
                            xtail[:, ct, d, :],
                            start=(i == 0),
                            stop=(i == n_acc - 1),
                        )
                        i += 1
                obt = opool.tile([128, bpc], DT.float32)
                nc.vector.tensor_scalar_add(obt[:], pst[:], bias_t[:, ot : ot + 1])
                nc.sync.dma_start(
                    y_d[:, ot * 128 : (ot + 1) * 128, Lc].transpose([1, 0]), obt[:]
                )

    nc.compile()
    return nc


def kernel(input, weight, P, bias):
    input = np.ascontiguousarray(input, np.float32)
    D = build_dense_kernel(weight, P)  # [O, C, LD]
    # D.reshape axes: [ot, o, ct, c, d] -> dw[c, d, ct, ot, o]
    dw = np.ascontiguousarray(D.reshape(2, 128, 2, 128, LD).transpose(3, 4, 2, 0, 1))
    bias2 = np.ascontiguousarray(
        np.asarray(bias, np.float32).reshape(2, 128).T
    )  # [128, 2]

    if "nc" not in _nc_cache:
        _nc_cache["nc"] = build_nc()
    nc = _nc_cache["nc"]

    zp = np.zeros((128, 2, PAD), np.float32)
    in_maps = [
        {
            "x": np.ascontiguousarray(input[i * BPC : (i + 1) * BPC]),
            "dw": dw,
            "bias": bias2,
            "zp": zp,
        }
        for i in range(NCORES)
    ]
    res = run_bass_kernel_spmd(nc, in_maps, core_ids=list(range(NCORES)))
    out = np.concatenate([r["y"] for r in res.results], axis=0)
    return out


# revision 14
# speedup vs baseline: 1.0043x; 1.0043x over previous
"""Dcls1d (dilated conv with learnable spacings) on 8 Trainium2 NeuronCores.

Problem (hardcoded): input [32, 256, 4096] f32, weight [256, 256, 7] f32,
P [256, 256, 7] f32, bias [256] f32 -> output [32, 256, 4097] f32.
The 7 taps are scattered at continuous positions into a dense 56-wide
kernel with linear interpolation (done host-side, bit-identical to the
reference fp32 math), then the dense conv runs on-device as 56 shifted
[128x128]x[128x512] fp32r matmuls accumulating in PSUM.

Sharding: data-parallel over batch — each of the 8 cores gets 4 batches,
weights/bias broadcast. No collectives; outputs concatenated on host.
"""

import os
from contextlib import ExitStack

import numpy as np

import concourse.bacc as bacc
import concourse.mybir as mybir
import concourse.tile as tile
from concourse.bass_utils import run_bass_kernel_spmd

DT = mybir.dt

B, CIN, COUT, L = 32, 256, 256, 4096
KTAPS, DIL, PAD = 7, 8, 28
LD = KTAPS * DIL  # 56 dense kernel width
TOUT = L + 2 * PAD - LD + 1  # 4097
NCORES = 8
BPC = B // NCORES  # batches per core

NT = 512  # output cols per psum tile
NCHUNK = L // NT  # 8 full chunks
CH_W = NT + LD  # 568 xpad cols per chunk

_nc_cache = {}


def build_dense_kernel(weight: np.ndarray, P: np.ndarray) -> np.ndarray:
    """Scatter taps into dense [O, C, LD] kernel. Replicates the reference's
    fp32 arithmetic exactly (clip/floor/frac all in float32)."""
    w = weight.astype(np.float32)
    pos = np.clip(P.astype(np.float32) + np.float32(LD // 2), np.float32(0.0), np.float32(LD - 1))
    lo = np.floor(pos)
    frac = pos - lo
    lo_i = lo.astype(np.int64)
    hi_i = np.minimum(lo_i + 1, LD - 1)
    O, C, K = w.shape
    oi = np.arange(O)[:, None, None]
    ci = np.arange(C)[None, :, None]
    D = np.zeros((O, C, LD), np.float32)
    np.add.at(D, (oi, ci, lo_i), w * (np.float32(1.0) - frac))
    np.add.at(D, (oi, ci, hi_i), w * frac)
    return D


def round_e8m11(a: np.ndarray) -> np.ndarray:
    """Round fp32 to the PE's fp32r operand format (e8m11 in the top 20
    bits). The PE truncates fp32 operands to 11 mantissa bits; pre-rounding
    to nearest-even halves that quantization error."""
    bits = np.ascontiguousarray(a, np.float32).view(np.uint32)
    out = (bits + np.uint32(0x7FF) + ((bits >> np.uint32(12)) & np.uint32(1))) & np.uint32(
        0xFFFFF000
    )
    return out.view(np.float32)


def build_nc(bpc=BPC, nchunk=NCHUNK):
    """Per-core program: conv of [bpc, 256, L'] with dense kernel."""
    Lc = nchunk * NT
    tout = Lc + 1 if nchunk == NCHUNK else Lc  # tail col only for full length

    nc = bacc.Bacc("TRN2", target_bir_lowering=False, debug=False)
    x_d = nc.dram_tensor("x", [bpc, CIN, Lc], DT.float32, kind="ExternalInput").ap()
    dw_d = nc.dram_tensor("dw", [128, LD, 2, 2, 128], DT.float32, kind="ExternalInput").ap()
    bias_d = nc.dram_tensor("bias", [128, 2], DT.float32, kind="ExternalInput").ap()
    zp_d = nc.dram_tensor("zp", [128, 2, PAD], DT.float32, kind="ExternalInput").ap()
    y_d = nc.dram_tensor("y", [bpc, COUT, tout], DT.float32, kind="ExternalOutput").ap()

    with ExitStack() as ctx:
        tc = ctx.enter_context(tile.TileContext(nc))
        wpool = ctx.enter_context(tc.tile_pool(name="w", bufs=1))
        xpool = ctx.enter_context(tc.tile_pool(name="x", bufs=3))
        opool = ctx.enter_context(tc.tile_pool(name="o", bufs=4))
        cpool = ctx.enter_context(tc.tile_pool(name="c", bufs=1))
        pspool = ctx.enter_context(tc.tile_pool(name="ps", bufs=4, space="PSUM"))
        tpool = ctx.enter_context(tc.tile_pool(name="tl", bufs=2, space="PSUM"))

        # split the 14.7MB weight DMA into d-range pieces so early matmuls
        # start as soon as their slice lands (one big DMA = ~50us dead start)
        DSPLIT = 8
        dchunk = LD // DSPLIT  # 7
        dw_tiles = []
        for i in range(DSPLIT):
            t = wpool.tile(
                [128, dchunk, 2, 2, 128], DT.float32r, name=f"dw{i}", tag=f"dw{i}"
            )
            nc.sync.dma_start(
                t[:], dw_d[:, i * dchunk : (i + 1) * dchunk].bitcast(DT.float32r)
            )
            dw_tiles.append(t)

        def dw_ap(d, ct, ot):
            return dw_tiles[d // dchunk][:, d % dchunk, ct, ot, :]

        bias_t = cpool.tile([128, 2], DT.float32)
        nc.sync.dma_start(bias_t[:], bias_d[:])
        # tail-column staging: x cols needed for output col t=Lc across batches
        xtail = cpool.tile([128, 2, PAD, bpc], DT.float32r)

        for b in range(bpc):
            for j in range(nchunk):
                t0 = NT * j
                # chunk holds xpad cols [t0, t0+CH_W); real data occupies
                # chunk-local [lo_real, hi_real), zeros DMA'd outside it
                lo_real = PAD if j == 0 else 0
                hi_real = min(CH_W, Lc - t0 + PAD)
                xc = xpool.tile([128, 2, CH_W], DT.float32r)
                if lo_real:
                    nc.sync.dma_start(
                        xc[:, :, 0:lo_real], zp_d[:].bitcast(DT.float32r)
                    )
                if hi_real < CH_W:
                    nc.sync.dma_start(
                        xc[:, :, hi_real:CH_W], zp_d[:].bitcast(DT.float32r)
                    )
                for ct in range(2):
                    nc.sync.dma_start(
                        xc[:, ct, lo_real:hi_real],
                        x_d[
                            b,
                            ct * 128 : (ct + 1) * 128,
                            t0 - PAD + lo_real : t0 - PAD + hi_real,
                        ].bitcast(DT.float32r),
                    )
                if tout != Lc and j == nchunk - 1:
                    # stash x cols [Lc-PAD, Lc) (chunk-local [NT, NT+PAD)) for
                    # the tail column t=Lc (computed interleaved below)
                    nc.vector.tensor_copy(
                        xtail[:, :, :, b], xc[:, :, NT : NT + PAD]
                    )
                # tail-column matmuls ride inside the last batch's last-chunk
                # groups, offset by 8 taps so their weight loads hide under the
                # big matmuls' streaming and the xtail stash has slack
                tail_here = tout != Lc and b == bpc - 1 and j == nchunk - 1
                TOFF = 8
                for ot in range(2):
                    ps = pspool.tile([128, NT], DT.float32)
                    pst = (
                        tpool.tile([128, bpc], DT.float32, name="pst", tag="pst")
                        if tail_here
                        else None
                    )
                    n_acc = LD * 2
                    i = 0
                    it = 0
                    for d in range(LD):
                        for ct in range(2):
                            nc.tensor.matmul(
                                ps[:],
                                dw_ap(d, ct, ot),
                                xc[:, ct, d : d + NT],
                                start=(i == 0),
                                stop=(i == n_acc - 1),
                            )
                            i += 1
                            td = d - TOFF
                            if tail_here and 0 <= td < PAD:
                                # out col t=Lc: only taps td < PAD see data
                                nc.tensor.matmul(
                                    pst[:],
                                    dw_ap(td, ct, ot),
                                    xtail[:, ct, td, :],
                                    start=(it == 0),
                                    stop=(it == 2 * PAD - 1),
                                    skip_group_check=True,
                                )
                                it += 1
                    ob = opool.tile([128, NT], DT.float32)
                    nc.vector.tensor_scalar_add(ob[:], ps[:], bias_t[:, ot : ot + 1])
                    nc.sync.dma_start(
                        y_d[b, ot * 128 : (ot + 1) * 128, t0 : t0 + NT], ob[:]
                    )
                    if tail_here:
                        obt = opool.tile([128, bpc], DT.float32)
                        nc.vector.tensor_scalar_add(
                            obt[:], pst[:], bias_t[:, ot : ot + 1]
                        )
                        nc.sync.dma_start(
                            y_d[:, ot * 128 : (ot + 1) * 128, Lc].transpose([1, 0]),
                            obt[:],
                        )

    nc.compile()
    return nc


def kernel(input, weight, P, bias):
    input = round_e8m11(np.ascontiguousarray(input, np.float32))
    D = round_e8m11(build_dense_kernel(weight, P))  # [O, C, LD]
    # D.reshape axes: [ot, o, ct, c, d] -> dw[c, d, ct, ot, o]
    dw = np.ascontiguousarray(D.reshape(2, 128, 2, 128, LD).transpose(3, 4, 2, 0, 1))
    bias2 = np.ascontiguousarray(
        np.asarray(bias, np.float32).reshape(2, 128).T
    )  # [128, 2]

    if "nc" not in _nc_cache:
        _nc_cache["nc"] = build_nc()
    nc = _nc_cache["nc"]

    zp = np.zeros((128, 2, PAD), np.float32)
    in_maps = [
        {
            "x": np.ascontiguousarray(input[i * BPC : (i + 1) * BPC]),
            "dw": dw,
            "bias": bias2,
            "zp": zp,
        }
        for i in range(NCORES)
    ]
    res = run_bass_kernel_spmd(nc, in_maps, core_ids=list(range(NCORES)))
    out = np.concatenate([r["y"] for r in res.results], axis=0)
    return out


# revision 16
# speedup vs baseline: 1.0225x; 1.0181x over previous
"""Dcls1d (dilated conv with learnable spacings) on 8 Trainium2 NeuronCores.

Problem (hardcoded): input [32, 256, 4096] f32, weight [256, 256, 7] f32,
P [256, 256, 7] f32, bias [256] f32 -> output [32, 256, 4097] f32.
The 7 taps are scattered at continuous positions into a dense 56-wide
kernel with linear interpolation (done host-side, bit-identical to the
reference fp32 math), then the dense conv runs on-device as 56 shifted
[128x128]x[128x512] fp32r matmuls accumulating in PSUM.

Sharding: data-parallel over batch — each of the 8 cores gets 4 batches,
weights/bias broadcast. No collectives; outputs concatenated on host.
"""

import os
from contextlib import ExitStack

import numpy as np

import concourse.bacc as bacc
import concourse.mybir as mybir
import concourse.tile as tile
from concourse.bass_utils import run_bass_kernel_spmd

DT = mybir.dt

B, CIN, COUT, L = 32, 256, 256, 4096
KTAPS, DIL, PAD = 7, 8, 28
LD = KTAPS * DIL  # 56 dense kernel width
TOUT = L + 2 * PAD - LD + 1  # 4097
NCORES = 8
BPC = B // NCORES  # batches per core

NT = 512  # output cols per psum tile
NCHUNK = L // NT  # 8 full chunks
CH_W = NT + LD  # 568 xpad cols per chunk

_nc_cache = {}


def build_dense_kernel(weight: np.ndarray, P: np.ndarray) -> np.ndarray:
    """Scatter taps into dense [O, C, LD] kernel. Replicates the reference's
    fp32 arithmetic exactly (clip/floor/frac all in float32)."""
    w = weight.astype(np.float32)
    pos = np.clip(P.astype(np.float32) + np.float32(LD // 2), np.float32(0.0), np.float32(LD - 1))
    lo = np.floor(pos)
    frac = pos - lo
    lo_i = lo.astype(np.int64)
    hi_i = np.minimum(lo_i + 1, LD - 1)
    O, C, K = w.shape
    oi = np.arange(O)[:, None, None]
    ci = np.arange(C)[None, :, None]
    D = np.zeros((O, C, LD), np.float32)
    np.add.at(D, (oi, ci, lo_i), w * (np.float32(1.0) - frac))
    np.add.at(D, (oi, ci, hi_i), w * frac)
    return D


def round_e8m11(a: np.ndarray) -> np.ndarray:
    """Round fp32 to the PE's fp32r operand format (e8m11 in the top 20
    bits). The PE truncates fp32 operands to 11 mantissa bits; pre-rounding
    to nearest-even halves that quantization error."""
    bits = np.ascontiguousarray(a, np.float32).view(np.uint32)
    out = (bits + np.uint32(0x7FF) + ((bits >> np.uint32(12)) & np.uint32(1))) & np.uint32(
        0xFFFFF000
    )
    return out.view(np.float32)


def build_nc(bpc=BPC, nchunk=NCHUNK):
    """Per-core program: conv of [bpc, 256, L'] with dense kernel."""
    Lc = nchunk * NT
    tout = Lc + 1 if nchunk == NCHUNK else Lc  # tail col only for full length

    nc = bacc.Bacc("TRN2", target_bir_lowering=False, debug=False)
    x_d = nc.dram_tensor("x", [bpc, CIN, Lc], DT.float32, kind="ExternalInput").ap()
    dw_d = nc.dram_tensor("dw", [128, LD, 2, 2, 128], DT.float32, kind="ExternalInput").ap()
    bias_d = nc.dram_tensor("bias", [128, 2], DT.float32, kind="ExternalInput").ap()
    zp_d = nc.dram_tensor("zp", [128, 2, PAD], DT.float32, kind="ExternalInput").ap()
    y_d = nc.dram_tensor("y", [bpc, COUT, tout], DT.float32, kind="ExternalOutput").ap()

    with ExitStack() as ctx:
        tc = ctx.enter_context(tile.TileContext(nc))
        wpool = ctx.enter_context(tc.tile_pool(name="w", bufs=1))
        xpool = ctx.enter_context(tc.tile_pool(name="x", bufs=3))
        opool = ctx.enter_context(tc.tile_pool(name="o", bufs=4))
        cpool = ctx.enter_context(tc.tile_pool(name="c", bufs=1))
        pspool = ctx.enter_context(tc.tile_pool(name="ps", bufs=4, space="PSUM"))
        tpool = ctx.enter_context(tc.tile_pool(name="tl", bufs=2, space="PSUM"))

        # split the 14.7MB weight DMA into d-range pieces so early matmuls
        # start as soon as their slice lands (one big DMA = ~50us dead start)
        DSPLIT = 8
        dchunk = LD // DSPLIT  # 7
        dw_tiles = []
        for i in range(DSPLIT):
            t = wpool.tile(
                [128, dchunk, 2, 2, 128], DT.float32r, name=f"dw{i}", tag=f"dw{i}"
            )
            nc.sync.dma_start(
                t[:], dw_d[:, i * dchunk : (i + 1) * dchunk].bitcast(DT.float32r)
            )
            dw_tiles.append(t)

        def dw_ap(d, ct, ot):
            return dw_tiles[d // dchunk][:, d % dchunk, ct, ot, :]

        bias_t = cpool.tile([128, 2], DT.float32)
        nc.scalar.dma_start(bias_t[:], bias_d[:])
        # tail-column staging: x cols needed for output col t=Lc across batches
        xtail = cpool.tile([128, 2, PAD, bpc], DT.float32r)

        for b in range(bpc):
            for j in range(nchunk):
                t0 = NT * j
                # chunk holds xpad cols [t0, t0+CH_W); real data occupies
                # chunk-local [lo_real, hi_real), zeros DMA'd outside it
                lo_real = PAD if j == 0 else 0
                hi_real = min(CH_W, Lc - t0 + PAD)
                xc = xpool.tile([128, 2, CH_W], DT.float32r)
                if lo_real:
                    nc.scalar.dma_start(
                        xc[:, :, 0:lo_real], zp_d[:].bitcast(DT.float32r)
                    )
                if hi_real < CH_W:
                    nc.scalar.dma_start(
                        xc[:, :, hi_real:CH_W], zp_d[:].bitcast(DT.float32r)
                    )
                for ct in range(2):
                    nc.scalar.dma_start(
                        xc[:, ct, lo_real:hi_real],
                        x_d[
                            b,
                            ct * 128 : (ct + 1) * 128,
                            t0 - PAD + lo_real : t0 - PAD + hi_real,
                        ].bitcast(DT.float32r),
                    )
                if tout != Lc and j == nchunk - 1:
                    # stash x cols [Lc-PAD, Lc) (chunk-local [NT, NT+PAD)) for
                    # the tail column t=Lc (computed interleaved below)
                    nc.vector.tensor_copy(
                        xtail[:, :, :, b], xc[:, :, NT : NT + PAD]
                    )
                # tail-column matmuls ride inside the last batch's last-chunk
                # groups, offset by 8 taps so their weight loads hide under the
                # big matmuls' streaming and the xtail stash has slack
                tail_here = tout != Lc and b == bpc - 1 and j == nchunk - 1
                TOFF = 8
                for ot in range(2):
                    ps = pspool.tile([128, NT], DT.float32)
                    pst = (
                        tpool.tile([128, bpc], DT.float32, name="pst", tag="pst")
                        if tail_here
                        else None
                    )
                    n_acc = LD * 2
                    i = 0
                    it = 0
                    for d in range(LD):
                        for ct in range(2):
                            nc.tensor.matmul(
                                ps[:],
                                dw_ap(d, ct, ot),
                                xc[:, ct, d : d + NT],
                                start=(i == 0),
                                stop=(i == n_acc - 1),
                            )
                            i += 1
                            td = d - TOFF
                            if tail_here and 0 <= td < PAD:
                                # out col t=Lc: only taps td < PAD see data
                                nc.tensor.matmul(
                                    pst[:],
                                    dw_ap(td, ct, ot),
                                    xtail[:, ct, td, :],
                                    start=(it == 0),
                                    stop=(it == 2 * PAD - 1),
                                    skip_group_check=True,
                                )
                                it += 1
                    ob = opool.tile([128, NT], DT.float32)
                    nc.vector.tensor_scalar_add(ob[:], ps[:], bias_t[:, ot : ot + 1])
                    nc.gpsimd.dma_start(
                        y_d[b, ot * 128 : (ot + 1) * 128, t0 : t0 + NT], ob[:]
                    )
                    if tail_here:
                        obt = opool.tile([128, bpc], DT.float32)
                        nc.vector.tensor_scalar_add(
                            obt[:], pst[:], bias_t[:, ot : ot + 1]
                        )
                        nc.gpsimd.dma_start(
                            y_d[:, ot * 128 : (ot + 1) * 128, Lc].transpose([1, 0]),
                            obt[:],
                        )

    nc.compile()
    return nc


def kernel(input, weight, P, bias):
    # note: the PE rounds fp32r operands to e8m11 (RNE) in hardware, so no
    # host-side quantization is needed (verified: pre-rounding is a no-op)
    input = np.ascontiguousarray(input, np.float32)
    D = build_dense_kernel(weight, P)  # [O, C, LD]
    # D.reshape axes: [ot, o, ct, c, d] -> dw[c, d, ct, ot, o]
    dw = np.ascontiguousarray(D.reshape(2, 128, 2, 128, LD).transpose(3, 4, 2, 0, 1))
    bias2 = np.ascontiguousarray(
        np.asarray(bias, np.float32).reshape(2, 128).T
    )  # [128, 2]

    if "nc" not in _nc_cache:
        _nc_cache["nc"] = build_nc()
    nc = _nc_cache["nc"]

    zp = np.zeros((128, 2, PAD), np.float32)
    in_maps = [
        {
            "x": np.ascontiguousarray(input[i * BPC : (i + 1) * BPC]),
            "dw": dw,
            "bias": bias2,
            "zp": zp,
        }
        for i in range(NCORES)
    ]
    res = run_bass_kernel_spmd(nc, in_maps, core_ids=list(range(NCORES)))
    out = np.concatenate([r["y"] for r in res.results], axis=0)
    return out
